# revision 46
# baseline (speedup 1.0000x reference)
"""Trainium2 Bass kernel for nn_AttentionBlock (B=2, S=2048, D=1024, H=16).

Sharding: 8 cores = data-parallel over batch (2) x tensor-parallel over
head groups (4 heads per core).  Each core computes its 4 heads'
attention plus its slice of the qkv / out projections; the host sums the
4 per-batch partial outputs and adds b_out.

v2 changes vs the fp32r baseline:
  - fp16 end-to-end (inputs, weights, SBUF intermediates, output
    partials): halves DMA bytes and SBUF pressure at the same PE rate
    (1 cycle/row for fp16 and fp32r alike).  Verified absmax err 7e-4.
  - software-pipelined attention j-loop: QK(j+1) is emitted before
    PV(j), so the ACT exp(j) (~1 us) hides behind ~1.7 us of PE work
    instead of stalling the in-order PE queue.
  - output partials DMA'd as fp16 straight from the PSUM->SBUF copy.

Per-core layout (all matmuls fp16 operands, fp32 PSUM):
  - host passes x[b].T so the contraction dim (d) is the partition dim
  - q,k computed transposed [e, s]; v computed natural [s, hd]
  - S^T[j,i] = k_h q_h^T, two heads packed in the PE array (row groups)
  - exp on ScalarE straight out of PSUM (scale = 1/8 folded into exp)
  - PV matmul with stationary [v_h | ones] -> unnormalized out^T plus the
    softmax row-sum in PSUM row 64, in one pass over E
  - normalize: reciprocal + gpsimd partition_broadcast + DVE multiply
  - final projection consumes the transposed attention output directly
"""

from contextlib import ExitStack
from functools import partial

import ml_dtypes
import numpy as np

import concourse.bass as bass
import concourse.tile as tile
from concourse import bacc, mybir
from concourse import bass_utils

B, S, D = 2, 2048, 1024
HD = 64          # head dim
HPC = 4          # heads per core
E_QK = 512       # q+k columns per core (2 * HPC * HD)
E_V = 256        # v columns per core
NCORES = 8

F32 = mybir.dt.float32
F16 = mybir.dt.float16

S_TILES = S // 128       # 16
D_TILES = D // 128       # 8
I_CHUNKS = S // 512      # 4 query chunks
J_TILES = S // 128       # 16 key tiles


def _make_pools(ctx, tc):
    return {
        "persist": ctx.enter_context(tc.tile_pool(name="persist", bufs=1)),
        "ps_s": ctx.enter_context(tc.tile_pool(name="ps_s", bufs=2, space="PSUM")),
        "ps_q": ctx.enter_context(tc.tile_pool(name="ps_q", bufs=2, space="PSUM")),
        "ps_pv": ctx.enter_context(tc.tile_pool(name="ps_pv", bufs=2, space="PSUM")),
        "epool": ctx.enter_context(tc.tile_pool(name="epool", bufs=4)),
        "spool": ctx.enter_context(tc.tile_pool(name="spool", bufs=2)),
        "rpool": ctx.enter_context(tc.tile_pool(name="rpool", bufs=2)),
        "evpool": ctx.enter_context(tc.tile_pool(name="evpool", bufs=3)),
    }


class _BufSet:
    """One alternation set of the persistent SBUF tensors.  Two sets let
    adjacent repetitions in the timing loop overlap with no WAR hazards."""

    def __init__(self, persist, i):
        self.xT_sb = persist.tile([128, D_TILES, S], F16, name=f"xT_sb{i}")
        self.w_qk_sb = persist.tile([128, D_TILES, E_QK], F16, name=f"w_qk_sb{i}")
        self.w_v_sb = persist.tile([128, D_TILES, E_V], F16, name=f"w_v_sb{i}")
        self.w_o_sb = persist.tile([128, 2, D], F16, name=f"w_o_sb{i}")
        self.b_qk_sb = persist.tile([128, 4], F32, name=f"b_qk_sb{i}")
        self.b_v_sb = persist.tile([1, E_V], F16, name=f"b_v_sb{i}")
        self.ones_sb = persist.tile([1, 128], F16, name=f"ones_sb{i}")
        self.qkT_sb = persist.tile([128, 4, S], F16, name=f"qkT_sb{i}")
        self.v_sb = persist.tile([128, S_TILES, HPC, HD + 1], F16, name=f"v_sb{i}")
        self.attnT_sb = persist.tile([128, 2, S], F16, name=f"attnT_sb{i}")
        # normalize gather: both heads' softmax row-sums at partitions 0/64;
        # rows 1-63 are DMA'd to 1.0 so the batched reciprocal never sees
        # garbage (suspected trigger of HW-only NaNs in an earlier attempt)
        self.s2_sb = persist.tile([65, 512], F32, name=f"s2_sb{i}")


def _build_nc(reps=1):
    nc = bacc.Bacc("TRN2", target_bir_lowering=False, debug=False, num_devices=NCORES)

    xT = nc.dram_tensor("xT", [D, S], F16, kind="ExternalInput")
    w_qk = nc.dram_tensor("w_qk", [D, E_QK], F16, kind="ExternalInput")
    w_v = nc.dram_tensor("w_v", [D, E_V], F16, kind="ExternalInput")
    w_o = nc.dram_tensor("w_o", [E_V, D], F16, kind="ExternalInput")
    b_qk = nc.dram_tensor("b_qk", [128, 4], F32, kind="ExternalInput")
    b_v = nc.dram_tensor("b_v", [1, E_V], F16, kind="ExternalInput")
    ones = nc.dram_tensor("ones", [128, 128], F16, kind="ExternalInput")
    onesf = nc.dram_tensor("onesf", [65, 512], F32, kind="ExternalInput")
    out = nc.dram_tensor("out", [S, D], F16, kind="ExternalOutput")
    aps = (xT.ap(), w_qk.ap(), w_v.ap(), w_o.ap(), b_qk.ap(), b_v.ap(), ones.ap(), onesf.ap(), out.ap())

    with tile.TileContext(nc) as tc, ExitStack() as ctx:
        pools = _make_pools(ctx, tc)
        s0 = _BufSet(pools["persist"], 0)
        if reps == 1:
            with ExitStack() as bctx:
                _body(bctx, tc, pools, s0, *aps)
        else:
            # two bodies per For_i iteration on alternating buffer sets:
            # the s1->s0 boundary inside an iteration pipelines freely (no
            # WAR between sets), and the For_i all-engine barrier cost is
            # paid once per two reps.
            assert reps % 2 == 1 and reps >= 3
            s1 = _BufSet(pools["persist"], 1)
            with ExitStack() as bctx:
                _body(bctx, tc, pools, s0, *aps)
            with tc.For_i(0, (reps - 1) // 2) as _i:
                with ExitStack() as b1:
                    _body(b1, tc, pools, s1, *aps)
                with ExitStack() as b2:
                    _body(b2, tc, pools, s0, *aps)
    nc.compile()
    return nc


def _body(ctx, tc, pools, T, xT, w_qk, w_v, w_o, b_qk, b_v, ones, onesf, out):
    from collections import deque

    nc = tc.nc
    Exp = mybir.ActivationFunctionType.Exp

    ps_s = pools["ps_s"]
    ps_q = pools["ps_q"]
    ps_pv = pools["ps_pv"]
    epool = pools["epool"]
    spool = pools["spool"]
    rpool = pools["rpool"]
    evpool = pools["evpool"]

    xT_sb = T.xT_sb
    w_qk_sb = T.w_qk_sb
    w_v_sb = T.w_v_sb
    w_o_sb = T.w_o_sb
    b_qk_sb = T.b_qk_sb
    b_v_sb = T.b_v_sb
    ones_sb = T.ones_sb
    qkT_sb = T.qkT_sb   # tiles 0-1: qT, 2-3: kT
    v_sb = T.v_sb
    attnT_sb = T.attnT_sb

    # ---- input DMAs: coalesced to ~14 large transfers (DMA issue costs
    # ~600ns of sequencer time each, so many small DMAs are issue-bound at
    # fp16 byte counts), ordered for the just-in-time schedule. ----
    xsl = lambda sc: slice(sc * 512, (sc + 1) * 512)
    xT_r = xT.rearrange("(t p) s -> p t s", p=128)      # [128, 8, S]
    w_qk_r = w_qk.rearrange("(t p) e -> p t e", p=128)  # [128, 8, 512]
    w_v_r = w_v.rearrange("(t p) e -> p t e", p=128)    # [128, 8, 256]
    w_o_r = w_o.rearrange("(t p) d -> p t d", p=128)    # [128, 2, 1024]
    nc.sync.dma_start(w_qk_sb[:, :, 256:384], w_qk_r[:, :, 256:384])  # k pair0
    nc.sync.dma_start(xT_sb[:, 0:4, xsl(0)], xT_r[:, 0:4, xsl(0)])
    nc.sync.dma_start(xT_sb[:, 4:8, xsl(0)], xT_r[:, 4:8, xsl(0)])
    nc.sync.dma_start(w_qk_sb[:, :, 0:128], w_qk_r[:, :, 0:128])      # q pair0
    nc.sync.dma_start(b_qk_sb[:], b_qk[:, :])
    nc.sync.dma_start(b_v_sb[:], b_v[:, :])
    nc.sync.dma_start(ones_sb[:], ones[0:1, 0:128])
    nc.sync.dma_start(v_sb[:, :, :, HD], ones[:, 0:64].rearrange("p (s h) -> p s h", s=S_TILES))
    nc.sync.dma_start(w_v_sb[:, :, :], w_v_r[:, :, :])
    nc.sync.dma_start(xT_sb[:, :, xsl(1)], xT_r[:, :, xsl(1)])
    nc.sync.dma_start(w_qk_sb[:, :, 384:512], w_qk_r[:, :, 384:512])  # k pair1
    nc.sync.dma_start(w_qk_sb[:, :, 128:256], w_qk_r[:, :, 128:256])  # q pair1
    nc.sync.dma_start(xT_sb[:, :, xsl(2)], xT_r[:, :, xsl(2)])
    nc.sync.dma_start(xT_sb[:, :, xsl(3)], xT_r[:, :, xsl(3)])
    nc.sync.dma_start(w_o_sb[:, :, :], w_o_r[:, :, :])
    nc.sync.dma_start(T.s2_sb[:], onesf[:, :])

    # ---- projection generators: yield ~2-matmul units (~200-430ns of PE
    # work) so they can be dribbled into the attention loop as PE filler ----
    def gen_qk(et, sc):
        psum = ps_q.tile([128, 512], F32, name="ps_qk", tag="psq")
        for d in range(D_TILES):
            nc.tensor.matmul(
                psum,
                (w_qk_sb[:, d, et * 128:(et + 1) * 128]),
                (xT_sb[:, d, xsl(sc)]),
                start=(d == 0), stop=(d == D_TILES - 1),
            )
            if d % 2 == 1 and d < D_TILES - 1:
                yield
        nc.vector.tensor_scalar_add(
            qkT_sb[:, et, xsl(sc)], psum, b_qk_sb[:, et:et + 1],
        )
        yield

    def gen_v(st):
        psum = ps_q.tile([128, 512], F32, name="ps_v", tag="psq")[:, :E_V]
        for d in range(D_TILES):
            nc.tensor.matmul(
                psum,
                (xT_sb[:, d, st * 128:(st + 1) * 128]),
                (w_v_sb[:, d, :]),
                start=(d == 0), stop=False,
            )
            if d % 2 == 1 and d < D_TILES - 1:
                yield
        # bias via rank-1 ones matmul (K=1)
        nc.tensor.matmul(psum, (ones_sb[:, :]), (b_v_sb[:, :]), start=False, stop=True)
        nc.vector.tensor_copy(
            v_sb[:, st, :, 0:HD],
            psum.rearrange("p (h e) -> p h e", h=HPC),
        )
        yield

    # output staging: one SBUF tile per query chunk, DMA'd to DRAM as a
    # single transfer once all 8 final-projection chains of the chunk wrote
    # into it (out-DMA issue time is ~600ns each; 4 beats 24).
    out_r = out.rearrange("(ic st p) (mc c) -> ic p st mc c", st=4, p=128, mc=2)
    o_chunk = [None] * I_CHUNKS

    def gen_final(st, mc, last=False, alt_pool=False):
        ic, sti = divmod(st, 4)
        msl = slice(mc * 512, (mc + 1) * 512)
        if alt_pool:
            # tail chains draw half their PSUM from the (idle) QK pool so
            # the next body's projection chains see a short psq WAR queue
            psF = ps_s.tile([128, 1024], F32, name="ps_fs", tag="pss")[:, 0:512]
        else:
            psF = ps_q.tile([128, 512], F32, name="ps_f", tag="psq")
        for kk in range(2):
            nc.tensor.matmul(
                psF,
                (attnT_sb[:, kk, st * 128:(st + 1) * 128]),
                (w_o_sb[:, kk, msl]),
                start=(kk == 0), stop=(kk == 1),
            )
        if o_chunk[ic] is None:
            o_chunk[ic] = spool.tile([128, 4, 2, 512], F16, name="o_chunk")
        if last:
            nc.scalar.copy(o_chunk[ic][:, sti, mc, :], psF)
        else:
            nc.vector.tensor_copy(o_chunk[ic][:, sti, mc, :], psF)
        yield

    def emit_out_dma(ic):
        # out-DMAs issue from the ACT engine's hardware DGE ring: the SP
        # queue parks on each out-DMA's data semaphore while issuing, which
        # would hold up the NEXT body's input-DMA issues (SP is in-order).
        # ACT is idle at chunk boundaries, where these land.
        nc.scalar.dma_start(out_r[ic], o_chunk[ic][:])

    filler = deque()

    def tick(n):
        """Advance the front filler generator by up to n yield-units."""
        while n > 0 and filler:
            try:
                next(filler[0])
                n -= 1
            except StopIteration:
                filler.popleft()

    def drain(gen):
        for _ in gen:
            pass

    def emit_attention(ic, pair, ticks_per_j, no_evac=False, tick_from=1):
        """Software-pipelined over j: QK(j+1) and filler are emitted before
        PV(j), so exp(j) (~1us on ACT) hides behind >=1.7us of PE work and
        the in-order PE queue never stalls on the Activation engine."""
        isl = slice(ic * 512, (ic + 1) * 512)
        pvA = ps_pv.tile([HD + 1, 512], F32, name="pvA", tag="pv")
        pvB = ps_pv.tile([HD + 1, 512], F32, name="pvB", tag="pv")

        e_t = [None] * J_TILES

        def emit_qk_j(j):
            jsl = slice(j * 128, (j + 1) * 128)
            psS = ps_s.tile([128, 1024], F32, name="psS", tag="pss")
            nc.tensor.matmul(
                psS[:, 0:512],
                (qkT_sb[0:64, 2 + pair, jsl]),
                (qkT_sb[0:64, pair, isl]),
                start=True, stop=True, tile_position=(0, 0),
            )
            nc.tensor.matmul(
                psS[:, 512:1024],
                (qkT_sb[64:128, 2 + pair, jsl]),
                (qkT_sb[64:128, pair, isl]),
                start=True, stop=True, tile_position=(64, 0),
            )
            e_t[j] = epool.tile([128, 1024], F16, name="e_t")
            nc.scalar.activation(e_t[j][:], psS[:], Exp, scale=0.125)

        def emit_pv_j(j):
            nc.tensor.matmul(
                pvA[:], (v_sb[:, j, 2 * pair, :]), (e_t[j][:, 0:512]),
                start=(j == 0), stop=(j == J_TILES - 1),
            )
            nc.tensor.matmul(
                pvB[:], (v_sb[:, j, 2 * pair + 1, :]), (e_t[j][:, 512:1024]),
                start=(j == 0), stop=(j == J_TILES - 1),
            )
            e_t[j] = None

        emit_qk_j(0)
        for j in range(1, J_TILES):
            emit_qk_j(j)
            if j >= tick_from:
                tick(ticks_per_j)
            emit_pv_j(j - 1)
        tick(ticks_per_j)
        emit_pv_j(J_TILES - 1)

        # evacuate BOTH PV psum banks first (fast DVE copies) so the next
        # pair's accumulation chains never wait on the normalize tail.  Both
        # heads' exp row-sums are gathered at partitions 0/64 of s2 (whose
        # other rows are DMA'd 1.0) so ONE reciprocal covers both heads:
        # the HW reciprocal runs ~6 cycles per free-dim element, so batching
        # partitions halves its latency.  The normalize multiplies run on
        # the otherwise-idle Pool engine to keep the DVE queue short at
        # chunk boundaries.
        ev = {}
        for h_loc, pv in ((0, pvA), (1, pvB)):
            nc.vector.tensor_copy(T.s2_sb[h_loc * 64:h_loc * 64 + 1, :], pv[HD:HD + 1, :])
            if no_evac:
                # last pair: nobody reuses the banks; skip the copy latency
                ev[h_loc] = pv
            else:
                evt = evpool.tile([HD, 512], F32, name="pv_ev", tag="pvev")
                nc.vector.tensor_copy(evt[:], pv[0:HD, :])
                ev[h_loc] = evt
        rec2 = rpool.tile([65, 512], F32, name="rec2", tag="rec")
        nc.vector.reciprocal(rec2[:], T.s2_sb[:])
        # HW gpsimd partition_broadcast only reads partition-0 sources (a
        # partition-64 source returns garbage on HW while CoreSim models it
        # fine) -- stage head B's reciprocal row through a partition-0 tile.
        recB = rpool.tile([1, 512], F32, name="recB", tag="recB")
        nc.vector.tensor_copy(recB[:], rec2[64:65, :])
        for h_loc, rsrc in ((0, rec2[0:1, :]), (1, recB[:])):
            rb = rpool.tile([HD, 512], F32, name="rb", tag="rb")
            nc.gpsimd.partition_broadcast(rb[:], rsrc)
            nc.vector.tensor_mul(
                attnT_sb[h_loc * 64:(h_loc + 1) * 64, pair, isl],
                ev[h_loc][0:HD, :],
                rb[:],
            )

    # ---- emission: a short prologue computes just enough (k0 sc0, q0 sc0,
    # v st0) for attention (0,0) to start ~7us in; every other projection is
    # dribbled into the attention j-loops as PE filler, in dependency order.
    # QK(ic,p,j) needs k_p sc(j//4) + q_p sc(ic); PV(j) needs v st(j). ----
    drain(gen_qk(2, 0))   # k pair0 sc0
    drain(gen_qk(0, 0))   # q pair0 sc0
    drain(gen_v(0))

    # (0,0): the forced filler (all v chains, rest of k0) plus k1/q1 sc0 so
    # attention (0,1) can start unstalled right after.
    filler.extend([gen_v(1), gen_v(2), gen_v(3), gen_qk(2, 1),
                   gen_v(4), gen_v(5), gen_v(6), gen_qk(2, 2),
                   gen_v(7), gen_v(8), gen_v(9), gen_qk(2, 3),
                   gen_v(10), gen_v(11), gen_v(12), gen_v(13),
                   gen_v(14), gen_v(15),
                   gen_qk(3, 0), gen_qk(1, 0)])
    emit_attention(0, 0, ticks_per_j=7)
    filler.extend([gen_qk(3, 1), gen_qk(3, 2), gen_qk(3, 3), gen_qk(0, 1)])
    emit_attention(0, 1, ticks_per_j=2)
    # remaining q projections ride along the chunk that precedes their use
    late_q = {(1, 0): gen_qk(1, 1), (1, 1): gen_qk(0, 2),
              (2, 0): gen_qk(1, 2), (2, 1): gen_qk(0, 3),
              (3, 0): gen_qk(1, 3)}
    for ic in range(1, I_CHUNKS):
        for pair in range(2):
            for st in range(ic * 4 - 4 + 2 * pair, ic * 4 - 2 + 2 * pair):
                filler.append(gen_final(st, 0))
                filler.append(gen_final(st, 1))
            if (ic, pair) in late_q:
                filler.append(late_q[(ic, pair)])
            # final-projection fillers start at j=7: by then the previous
            # chunk's normalize chain (gather + batched reciprocal +
            # broadcast + Pool mul, ~6.5us) has produced the attnT rows
            # they need, so the in-order PE never parks on a kk=1 matmul.
            emit_attention(ic, pair, ticks_per_j=2, tick_from=7,
                           no_evac=(ic == I_CHUNKS - 1 and pair == 1))
        while filler:   # chunk ic-1's chains all queued by now; force-drain
            tick(1000)
        emit_out_dma(ic - 1)
    # tail: alternate PSUM evacuations between ScalarE and DVE so the 8
    # remaining chains drain on two engines; DMA the chunk in two halves so
    # the first transfer overlaps the second half's chains.
    for i, st in enumerate(range(12, 16)):
        for mc in range(2):
            drain(gen_final(st, mc, last=(i + mc) % 2 == 0, alt_pool=mc == 1))
        if st == 13:
            nc.scalar.dma_start(out_r[3][:, 0:2], o_chunk[3][:, 0:2])
    nc.scalar.dma_start(out_r[3][:, 2:4], o_chunk[3][:, 2:4])


_CACHE = {}


def _get_nc(reps=1):
    key = ("nc", reps)
    if key not in _CACHE:
        _CACHE[key] = _build_nc(reps)
    return _CACHE[key]


def _get_runner(reps=1):
    """Build (once) a jitted shard_map executable over the 8 cores.

    Mirrors bass2jax.run_bass_via_pjrt but caches the jitted function so
    repeat kernel() calls and benchmarking skip retrace/recompile.
    """
    if ("runner", reps) in _CACHE:
        return _CACHE[("runner", reps)]
    import jax
    import jax.numpy as jnp
    from jax.sharding import Mesh, PartitionSpec
    from jax.experimental.shard_map import shard_map
    from concourse import bass2jax

    nc = _get_nc(reps)
    bass2jax.install_neuronx_cc_hook()

    partition_name = nc.partition_id_tensor.name if nc.partition_id_tensor else None
    in_names, out_names, out_avals = [], [], []
    for alloc in nc.m.functions[0].allocations:
        if not isinstance(alloc, mybir.MemoryLocationSet):
            continue
        name = alloc.memorylocations[0].name
        if alloc.kind == "ExternalInput":
            if name != partition_name:
                in_names.append(name)
        elif alloc.kind == "ExternalOutput":
            shape = tuple(alloc.tensor_shape)
            dtype = mybir.dt.np(alloc.dtype)
            out_names.append(name)
            out_avals.append(jax.core.ShapedArray(shape, dtype))
    n_params = len(in_names)
    n_outs = len(out_avals)
    all_names = in_names + out_names
    if partition_name is not None:
        all_names = all_names + [partition_name]
    donate = tuple(range(n_params, n_params + n_outs))

    def _jit_body(*args):
        operands = list(args)
        if partition_name is not None:
            operands.append(bass2jax.partition_id_tensor())
        outs = bass2jax._bass_exec_p.bind(
            *operands,
            out_avals=tuple(out_avals),
            in_names=tuple(all_names),
            out_names=tuple(out_names),
            lowering_input_output_aliases=(),
            sim_require_finite=True,
            sim_require_nnan=True,
            nc=nc,
        )
        return tuple(outs)

    devices = jax.devices()[:NCORES]
    mesh = Mesh(np.asarray(devices), ("core",))
    sharded = jax.jit(
        shard_map(
            _jit_body, mesh=mesh,
            in_specs=(PartitionSpec("core"),) * (n_params + n_outs),
            out_specs=(PartitionSpec("core"),) * n_outs,
            check_rep=False,
        ),
        donate_argnums=donate, keep_unused=True,
    )

    from jax.sharding import NamedSharding
    core_sharding = NamedSharding(mesh, PartitionSpec("core"))

    @partial(jax.jit, out_shardings=core_sharding)
    def _zeros():
        return tuple(
            jnp.zeros((NCORES * a.shape[0],) + a.shape[1:], a.dtype)
            for a in out_avals)

    def run(in_maps, device_arrays=None, timeit=False):
        import time as _time
        if device_arrays is None:
            concat_in = [
                np.concatenate([np.asarray(m[name]) for m in in_maps], axis=0)
                for name in in_names]
            device_arrays = [jax.device_put(a, core_sharding) for a in concat_in]
        zs = jax.block_until_ready(_zeros())
        t0 = _time.perf_counter()
        out_arrs = jax.block_until_ready(sharded(*device_arrays, *zs))
        dt = _time.perf_counter() - t0
        results = [
            {name: np.asarray(out_arrs[i]).reshape(NCORES, *out_avals[i].shape)[c]
             for i, name in enumerate(out_names)}
            for c in range(NCORES)]
        if timeit:
            return results, dt, device_arrays
        return results

    def bench(in_maps, iters=10, batches=3):
        """Pipelined timing: dispatch `iters` executions back-to-back and
        block once, amortizing the per-dispatch RPC latency.  Returns the
        min per-iter average across `batches` batches."""
        import time as _time
        concat_in = [
            np.concatenate([np.asarray(m[name]) for m in in_maps], axis=0)
            for name in in_names]
        device_arrays = [jax.device_put(a, core_sharding) for a in concat_in]
        best = None
        for b in range(batches):
            all_zs = [jax.block_until_ready(_zeros()) for _ in range(iters + 1)]
            jax.block_until_ready(sharded(*device_arrays, *all_zs[0]))
            t0 = _time.perf_counter()
            outs = [sharded(*device_arrays, *all_zs[1 + i]) for i in range(iters)]
            jax.block_until_ready(outs)
            dt = (_time.perf_counter() - t0) / iters
            best = dt if best is None else min(best, dt)
        return best

    _CACHE[("bench", reps)] = bench
    _CACHE[("runner", reps)] = run
    return run


def _core_inputs(x, w_qkv, b_qkv, w_out):
    """Host-side sharding: returns the 8 per-core input dicts (fp16)."""
    f16 = np.float16
    in_maps = []
    for c in range(NCORES):
        b, g = divmod(c, 4)
        e0 = g * HPC * HD  # first column of this core's head group
        q_cols = slice(e0, e0 + E_V)
        k_cols = slice(D + e0, D + e0 + E_V)
        v_cols = slice(2 * D + e0, 2 * D + e0 + E_V)
        w_qk_c = np.ascontiguousarray(
            np.concatenate([w_qkv[:, q_cols], w_qkv[:, k_cols]], axis=1)).astype(f16)
        b_qk_c = np.ascontiguousarray(
            np.concatenate([b_qkv[q_cols], b_qkv[k_cols]]).reshape(4, 128).T)
        in_maps.append({
            "xT": np.ascontiguousarray(x[b].T).astype(f16),
            "w_qk": w_qk_c,
            "w_v": np.ascontiguousarray(w_qkv[:, v_cols]).astype(f16),
            "w_o": np.ascontiguousarray(w_out[e0:e0 + E_V, :]).astype(f16),
            "b_qk": b_qk_c.astype(np.float32),
            "b_v": np.ascontiguousarray(b_qkv[v_cols]).reshape(1, E_V).astype(f16),
            "ones": np.ones((128, 128), dtype=f16),
            "onesf": np.ones((65, 512), dtype=np.float32),
        })
    return in_maps


def kernel(x, w_qkv, b_qkv, w_out, b_out):
    x = np.asarray(x, dtype=np.float32)
    w_qkv = np.asarray(w_qkv, dtype=np.float32)
    b_qkv = np.asarray(b_qkv, dtype=np.float32)
    w_out = np.asarray(w_out, dtype=np.float32)
    b_out = np.asarray(b_out, dtype=np.float32)

    run = _get_runner()
    in_maps = _core_inputs(x, w_qkv, b_qkv, w_out)
    results = run(in_maps)
    partials = np.stack([results[c]["out"].astype(np.float32) for c in range(NCORES)])
    full = partials.reshape(B, 4, S, D).sum(axis=1) + b_out
    return full.astype(np.float32)


# revision 48
# speedup vs baseline: 1.2486x; 1.2486x over previous
"""Trainium2 Bass kernel for nn_AttentionBlock (B=2, S=2048, D=1024, H=16).

Sharding: 8 cores = data-parallel over batch (2) x tensor-parallel over
head groups (4 heads per core).  Each core computes its 4 heads'
attention plus its slice of the qkv / out projections; the host sums the
4 per-batch partial outputs and adds b_out.

v2 changes vs the fp32r baseline:
  - fp16 end-to-end (inputs, weights, SBUF intermediates, output
    partials): halves DMA bytes and SBUF pressure at the same PE rate
    (1 cycle/row for fp16 and fp32r alike).  Verified absmax err 7e-4.
  - software-pipelined attention j-loop: QK(j+1) is emitted before
    PV(j), so the ACT exp(j) (~1 us) hides behind ~1.7 us of PE work
    instead of stalling the in-order PE queue.
  - output partials DMA'd as fp16 straight from the PSUM->SBUF copy.

Per-core layout (all matmuls fp16 operands, fp32 PSUM):
  - host passes x[b].T so the contraction dim (d) is the partition dim
  - q,k computed transposed [e, s]; v computed natural [s, hd]
  - S^T[j,i] = k_h q_h^T, two heads packed in the PE array (row groups)
  - exp on ScalarE straight out of PSUM (scale = 1/8 folded into exp)
  - PV matmul with stationary [v_h | ones] -> unnormalized out^T plus the
    softmax row-sum in PSUM row 64, in one pass over E
  - normalize: reciprocal + gpsimd partition_broadcast + DVE multiply
  - final projection consumes the transposed attention output directly
"""

from contextlib import ExitStack
from functools import partial

import ml_dtypes
import numpy as np

import concourse.bass as bass
import concourse.tile as tile
from concourse import bacc, mybir
from concourse import bass_utils

B, S, D = 2, 2048, 1024
HD = 64          # head dim
HPC = 4          # heads per core
E_QK = 512       # q+k columns per core (2 * HPC * HD)
E_V = 256        # v columns per core
NCORES = 8

F32 = mybir.dt.float32
F16 = mybir.dt.float16

S_TILES = S // 128       # 16
D_TILES = D // 128       # 8
I_CHUNKS = S // 512      # 4 query chunks
J_TILES = S // 128       # 16 key tiles


def _make_pools(ctx, tc):
    return {
        "persist": ctx.enter_context(tc.tile_pool(name="persist", bufs=1)),
        "ps_s": ctx.enter_context(tc.tile_pool(name="ps_s", bufs=2, space="PSUM")),
        "ps_q": ctx.enter_context(tc.tile_pool(name="ps_q", bufs=2, space="PSUM")),
        "ps_pv": ctx.enter_context(tc.tile_pool(name="ps_pv", bufs=2, space="PSUM")),
        "epool": ctx.enter_context(tc.tile_pool(name="epool", bufs=4)),
        "spool": ctx.enter_context(tc.tile_pool(name="spool", bufs=2)),
        "rpool": ctx.enter_context(tc.tile_pool(name="rpool", bufs=2)),
        "evpool": ctx.enter_context(tc.tile_pool(name="evpool", bufs=3)),
    }


class _BufSet:
    """One alternation set of the persistent SBUF tensors.  Two sets let
    adjacent repetitions in the timing loop overlap with no WAR hazards."""

    def __init__(self, persist, i):
        self.xT_sb = persist.tile([128, D_TILES, S], F16, name=f"xT_sb{i}")
        self.w_qk_sb = persist.tile([128, D_TILES, E_QK], F16, name=f"w_qk_sb{i}")
        self.w_v_sb = persist.tile([128, D_TILES, E_V], F16, name=f"w_v_sb{i}")
        self.w_o_sb = persist.tile([128, 2, D], F16, name=f"w_o_sb{i}")
        self.b_qk_sb = persist.tile([128, 4], F32, name=f"b_qk_sb{i}")
        self.b_v_sb = persist.tile([1, E_V], F16, name=f"b_v_sb{i}")
        self.ones_sb = persist.tile([1, 128], F16, name=f"ones_sb{i}")
        self.qkT_sb = persist.tile([128, 4, S], F16, name=f"qkT_sb{i}")
        self.v_sb = persist.tile([128, S_TILES, HPC, HD + 1], F16, name=f"v_sb{i}")
        self.attnT_sb = persist.tile([128, 2, S], F16, name=f"attnT_sb{i}")
        # normalize gather: both heads' softmax row-sums at partitions 0/64;
        # rows 1-63 are DMA'd to 1.0 so the batched reciprocal never sees
        # garbage (suspected trigger of HW-only NaNs in an earlier attempt)
        self.s2_sb = persist.tile([65, 512], F32, name=f"s2_sb{i}")


def _build_nc(reps=1):
    nc = bacc.Bacc("TRN2", target_bir_lowering=False, debug=False, num_devices=NCORES)

    xT = nc.dram_tensor("xT", [D, S], F16, kind="ExternalInput")
    w_qk = nc.dram_tensor("w_qk", [D, E_QK], F16, kind="ExternalInput")
    w_v = nc.dram_tensor("w_v", [D, E_V], F16, kind="ExternalInput")
    w_o = nc.dram_tensor("w_o", [E_V, D], F16, kind="ExternalInput")
    b_qk = nc.dram_tensor("b_qk", [128, 4], F32, kind="ExternalInput")
    b_v = nc.dram_tensor("b_v", [1, E_V], F16, kind="ExternalInput")
    ones = nc.dram_tensor("ones", [128, 128], F16, kind="ExternalInput")
    onesf = nc.dram_tensor("onesf", [65, 512], F32, kind="ExternalInput")
    out = nc.dram_tensor("out", [S, D], F16, kind="ExternalOutput")
    aps = (xT.ap(), w_qk.ap(), w_v.ap(), w_o.ap(), b_qk.ap(), b_v.ap(), ones.ap(), onesf.ap(), out.ap())

    with tile.TileContext(nc) as tc, ExitStack() as ctx:
        pools = _make_pools(ctx, tc)
        s0 = _BufSet(pools["persist"], 0)
        if reps == 1:
            with ExitStack() as bctx:
                _body(bctx, tc, pools, s0, *aps)
        else:
            # four bodies per For_i iteration on alternating buffer sets:
            # set-alternated boundaries inside an iteration pipeline freely
            # (no WAR between sets), and the For_i all-engine barrier (plus
            # the cold-restart prologue drip after it) is paid once per
            # FOUR reps instead of every rep.
            assert (reps - 1) % 4 == 0 and reps >= 5
            s1 = _BufSet(pools["persist"], 1)
            with ExitStack() as bctx:
                _body(bctx, tc, pools, s0, *aps)
            with tc.For_i(0, (reps - 1) // 4) as _i:
                for T in (s1, s0, s1, s0):
                    with ExitStack() as bctx2:
                        _body(bctx2, tc, pools, T, *aps)
    nc.compile()
    return nc


def _body(ctx, tc, pools, T, xT, w_qk, w_v, w_o, b_qk, b_v, ones, onesf, out):
    from collections import deque

    nc = tc.nc
    Exp = mybir.ActivationFunctionType.Exp

    ps_s = pools["ps_s"]
    ps_q = pools["ps_q"]
    ps_pv = pools["ps_pv"]
    epool = pools["epool"]
    spool = pools["spool"]
    rpool = pools["rpool"]
    evpool = pools["evpool"]

    xT_sb = T.xT_sb
    w_qk_sb = T.w_qk_sb
    w_v_sb = T.w_v_sb
    w_o_sb = T.w_o_sb
    b_qk_sb = T.b_qk_sb
    b_v_sb = T.b_v_sb
    ones_sb = T.ones_sb
    qkT_sb = T.qkT_sb   # tiles 0-1: qT, 2-3: kT
    v_sb = T.v_sb
    attnT_sb = T.attnT_sb

    # ---- input DMAs: coalesced to ~14 large transfers (DMA issue costs
    # ~600ns of sequencer time each, so many small DMAs are issue-bound at
    # fp16 byte counts), ordered for the just-in-time schedule. ----
    xsl = lambda sc: slice(sc * 512, (sc + 1) * 512)
    xT_r = xT.rearrange("(t p) s -> p t s", p=128)      # [128, 8, S]
    w_qk_r = w_qk.rearrange("(t p) e -> p t e", p=128)  # [128, 8, 512]
    w_v_r = w_v.rearrange("(t p) e -> p t e", p=128)    # [128, 8, 256]
    w_o_r = w_o.rearrange("(t p) d -> p t d", p=128)    # [128, 2, 1024]
    nc.sync.dma_start(w_qk_sb[:, :, 256:384], w_qk_r[:, :, 256:384])  # k pair0
    nc.sync.dma_start(xT_sb[:, 0:4, xsl(0)], xT_r[:, 0:4, xsl(0)])
    nc.sync.dma_start(xT_sb[:, 4:8, xsl(0)], xT_r[:, 4:8, xsl(0)])
    nc.sync.dma_start(w_qk_sb[:, :, 0:128], w_qk_r[:, :, 0:128])      # q pair0
    nc.sync.dma_start(b_qk_sb[:], b_qk[:, :])
    nc.sync.dma_start(b_v_sb[:], b_v[:, :])
    nc.sync.dma_start(ones_sb[:], ones[0:1, 0:128])
    nc.sync.dma_start(v_sb[:, :, :, HD], ones[:, 0:64].rearrange("p (s h) -> p s h", s=S_TILES))
    nc.sync.dma_start(w_v_sb[:, :, :], w_v_r[:, :, :])
    nc.sync.dma_start(xT_sb[:, :, xsl(1)], xT_r[:, :, xsl(1)])
    nc.sync.dma_start(w_qk_sb[:, :, 384:512], w_qk_r[:, :, 384:512])  # k pair1
    nc.sync.dma_start(w_qk_sb[:, :, 128:256], w_qk_r[:, :, 128:256])  # q pair1
    nc.sync.dma_start(xT_sb[:, :, xsl(2)], xT_r[:, :, xsl(2)])
    nc.sync.dma_start(xT_sb[:, :, xsl(3)], xT_r[:, :, xsl(3)])
    nc.sync.dma_start(w_o_sb[:, :, :], w_o_r[:, :, :])
    nc.sync.dma_start(T.s2_sb[:], onesf[:, :])

    # ---- projection generators: yield ~2-matmul units (~200-430ns of PE
    # work) so they can be dribbled into the attention loop as PE filler ----
    def gen_qk(et, sc):
        psum = ps_q.tile([128, 512], F32, name="ps_qk", tag="psq")
        for d in range(D_TILES):
            nc.tensor.matmul(
                psum,
                (w_qk_sb[:, d, et * 128:(et + 1) * 128]),
                (xT_sb[:, d, xsl(sc)]),
                start=(d == 0), stop=(d == D_TILES - 1),
            )
            if d % 2 == 1 and d < D_TILES - 1:
                yield
        nc.vector.tensor_scalar_add(
            qkT_sb[:, et, xsl(sc)], psum, b_qk_sb[:, et:et + 1],
        )
        yield

    def gen_v(st):
        psum = ps_q.tile([128, 512], F32, name="ps_v", tag="psq")[:, :E_V]
        for d in range(D_TILES):
            nc.tensor.matmul(
                psum,
                (xT_sb[:, d, st * 128:(st + 1) * 128]),
                (w_v_sb[:, d, :]),
                start=(d == 0), stop=False,
            )
            if d % 2 == 1 and d < D_TILES - 1:
                yield
        # bias via rank-1 ones matmul (K=1)
        nc.tensor.matmul(psum, (ones_sb[:, :]), (b_v_sb[:, :]), start=False, stop=True)
        nc.vector.tensor_copy(
            v_sb[:, st, :, 0:HD],
            psum.rearrange("p (h e) -> p h e", h=HPC),
        )
        yield

    # output staging: one SBUF tile per query chunk, DMA'd to DRAM as a
    # single transfer once all 8 final-projection chains of the chunk wrote
    # into it (out-DMA issue time is ~600ns each; 4 beats 24).
    out_r = out.rearrange("(ic st p) (mc c) -> ic p st mc c", st=4, p=128, mc=2)
    o_chunk = [None] * I_CHUNKS

    def gen_final(st, mc, last=False, alt_pool=False):
        ic, sti = divmod(st, 4)
        msl = slice(mc * 512, (mc + 1) * 512)
        if alt_pool:
            # tail chains draw half their PSUM from the (idle) QK pool so
            # the next body's projection chains see a short psq WAR queue
            psF = ps_s.tile([128, 1024], F32, name="ps_fs", tag="pss")[:, 0:512]
        else:
            psF = ps_q.tile([128, 512], F32, name="ps_f", tag="psq")
        for kk in range(2):
            nc.tensor.matmul(
                psF,
                (attnT_sb[:, kk, st * 128:(st + 1) * 128]),
                (w_o_sb[:, kk, msl]),
                start=(kk == 0), stop=(kk == 1),
            )
        if o_chunk[ic] is None:
            o_chunk[ic] = spool.tile([128, 4, 2, 512], F16, name="o_chunk")
        if last:
            nc.scalar.copy(o_chunk[ic][:, sti, mc, :], psF)
        else:
            nc.vector.tensor_copy(o_chunk[ic][:, sti, mc, :], psF)
        yield

    def emit_out_dma(ic):
        nc.sync.dma_start(out_r[ic], o_chunk[ic][:])

    filler = deque()

    def tick(n):
        """Advance the front filler generator by up to n yield-units."""
        while n > 0 and filler:
            try:
                next(filler[0])
                n -= 1
            except StopIteration:
                filler.popleft()

    def drain(gen):
        for _ in gen:
            pass

    def emit_attention(ic, pair, ticks_per_j, no_evac=False, tick_from=1):
        """Software-pipelined over j: QK(j+1) and filler are emitted before
        PV(j), so exp(j) (~1us on ACT) hides behind >=1.7us of PE work and
        the in-order PE queue never stalls on the Activation engine."""
        isl = slice(ic * 512, (ic + 1) * 512)
        pvA = ps_pv.tile([HD + 1, 512], F32, name="pvA", tag="pv")
        pvB = ps_pv.tile([HD + 1, 512], F32, name="pvB", tag="pv")

        e_t = [None] * J_TILES

        def emit_qk_j(j):
            jsl = slice(j * 128, (j + 1) * 128)
            psS = ps_s.tile([128, 1024], F32, name="psS", tag="pss")
            nc.tensor.matmul(
                psS[:, 0:512],
                (qkT_sb[0:64, 2 + pair, jsl]),
                (qkT_sb[0:64, pair, isl]),
                start=True, stop=True, tile_position=(0, 0),
            )
            nc.tensor.matmul(
                psS[:, 512:1024],
                (qkT_sb[64:128, 2 + pair, jsl]),
                (qkT_sb[64:128, pair, isl]),
                start=True, stop=True, tile_position=(64, 0),
            )
            e_t[j] = epool.tile([128, 1024], F16, name="e_t")
            nc.scalar.activation(e_t[j][:], psS[:], Exp, scale=0.125)

        def emit_pv_j(j):
            nc.tensor.matmul(
                pvA[:], (v_sb[:, j, 2 * pair, :]), (e_t[j][:, 0:512]),
                start=(j == 0), stop=(j == J_TILES - 1),
            )
            nc.tensor.matmul(
                pvB[:], (v_sb[:, j, 2 * pair + 1, :]), (e_t[j][:, 512:1024]),
                start=(j == 0), stop=(j == J_TILES - 1),
            )
            e_t[j] = None

        emit_qk_j(0)
        for j in range(1, J_TILES):
            emit_qk_j(j)
            if j >= tick_from:
                tick(ticks_per_j)
            emit_pv_j(j - 1)
        tick(ticks_per_j)
        emit_pv_j(J_TILES - 1)

        # evacuate BOTH PV psum banks first (fast DVE copies) so the next
        # pair's accumulation chains never wait on the normalize tail.  Both
        # heads' exp row-sums are gathered at partitions 0/64 of s2 (whose
        # other rows are DMA'd 1.0) so ONE reciprocal covers both heads:
        # the HW reciprocal runs ~6 cycles per free-dim element, so batching
        # partitions halves its latency.  The normalize multiplies run on
        # the otherwise-idle Pool engine to keep the DVE queue short at
        # chunk boundaries.
        ev = {}
        for h_loc, pv in ((0, pvA), (1, pvB)):
            nc.vector.tensor_copy(T.s2_sb[h_loc * 64:h_loc * 64 + 1, :], pv[HD:HD + 1, :])
            if no_evac:
                # last pair: nobody reuses the banks; skip the copy latency
                ev[h_loc] = pv
            else:
                evt = evpool.tile([HD, 512], F32, name="pv_ev", tag="pvev")
                nc.vector.tensor_copy(evt[:], pv[0:HD, :])
                ev[h_loc] = evt
        rec2 = rpool.tile([65, 512], F32, name="rec2", tag="rec")
        nc.vector.reciprocal(rec2[:], T.s2_sb[:])
        # HW gpsimd partition_broadcast only reads partition-0 sources (a
        # partition-64 source returns garbage on HW while CoreSim models it
        # fine) -- stage head B's reciprocal row through a partition-0 tile.
        recB = rpool.tile([1, 512], F32, name="recB", tag="recB")
        nc.vector.tensor_copy(recB[:], rec2[64:65, :])
        for h_loc, rsrc in ((0, rec2[0:1, :]), (1, recB[:])):
            rb = rpool.tile([HD, 512], F32, name="rb", tag="rb")
            nc.gpsimd.partition_broadcast(rb[:], rsrc)
            nc.vector.tensor_mul(
                attnT_sb[h_loc * 64:(h_loc + 1) * 64, pair, isl],
                ev[h_loc][0:HD, :],
                rb[:],
            )

    # ---- emission: a short prologue computes just enough (k0 sc0, q0 sc0,
    # v st0) for attention (0,0) to start ~7us in; every other projection is
    # dribbled into the attention j-loops as PE filler, in dependency order.
    # QK(ic,p,j) needs k_p sc(j//4) + q_p sc(ic); PV(j) needs v st(j). ----
    drain(gen_qk(2, 0))   # k pair0 sc0
    drain(gen_qk(0, 0))   # q pair0 sc0
    drain(gen_v(0))

    # (0,0): the forced filler (all v chains, rest of k0) plus k1/q1 sc0 so
    # attention (0,1) can start unstalled right after.
    filler.extend([gen_v(1), gen_v(2), gen_v(3), gen_qk(2, 1),
                   gen_v(4), gen_v(5), gen_v(6), gen_qk(2, 2),
                   gen_v(7), gen_v(8), gen_v(9), gen_qk(2, 3),
                   gen_v(10), gen_v(11), gen_v(12), gen_v(13),
                   gen_v(14), gen_v(15),
                   gen_qk(3, 0), gen_qk(1, 0)])
    emit_attention(0, 0, ticks_per_j=7)
    filler.extend([gen_qk(3, 1), gen_qk(3, 2), gen_qk(3, 3), gen_qk(0, 1)])
    emit_attention(0, 1, ticks_per_j=2)
    # remaining q projections ride along the chunk that precedes their use
    late_q = {(1, 0): gen_qk(1, 1), (1, 1): gen_qk(0, 2),
              (2, 0): gen_qk(1, 2), (2, 1): gen_qk(0, 3),
              (3, 0): gen_qk(1, 3)}
    for ic in range(1, I_CHUNKS):
        for pair in range(2):
            for st in range(ic * 4 - 4 + 2 * pair, ic * 4 - 2 + 2 * pair):
                filler.append(gen_final(st, 0))
                filler.append(gen_final(st, 1))
            if (ic, pair) in late_q:
                filler.append(late_q[(ic, pair)])
            # final-projection fillers start at j=7: by then the previous
            # chunk's normalize chain (gather + batched reciprocal +
            # broadcast + Pool mul, ~6.5us) has produced the attnT rows
            # they need, so the in-order PE never parks on a kk=1 matmul.
            emit_attention(ic, pair, ticks_per_j=2, tick_from=7,
                           no_evac=(ic == I_CHUNKS - 1 and pair == 1))
        while filler:   # chunk ic-1's chains all queued by now; force-drain
            tick(1000)
        emit_out_dma(ic - 1)
    # tail: alternate PSUM evacuations between ScalarE and DVE so the 8
    # remaining chains drain on two engines; DMA the chunk in two halves so
    # the first transfer overlaps the second half's chains.
    for i, st in enumerate(range(12, 16)):
        for mc in range(2):
            drain(gen_final(st, mc, last=(i + mc) % 2 == 0, alt_pool=mc == 1))
        if st == 13:
            nc.sync.dma_start(out_r[3][:, 0:2], o_chunk[3][:, 0:2])
    nc.sync.dma_start(out_r[3][:, 2:4], o_chunk[3][:, 2:4])


_CACHE = {}


def _get_nc(reps=1):
    key = ("nc", reps)
    if key not in _CACHE:
        _CACHE[key] = _build_nc(reps)
    return _CACHE[key]


def _get_runner(reps=1):
    """Build (once) a jitted shard_map executable over the 8 cores.

    Mirrors bass2jax.run_bass_via_pjrt but caches the jitted function so
    repeat kernel() calls and benchmarking skip retrace/recompile.
    """
    if ("runner", reps) in _CACHE:
        return _CACHE[("runner", reps)]
    import jax
    import jax.numpy as jnp
    from jax.sharding import Mesh, PartitionSpec
    from jax.experimental.shard_map import shard_map
    from concourse import bass2jax

    nc = _get_nc(reps)
    bass2jax.install_neuronx_cc_hook()

    partition_name = nc.partition_id_tensor.name if nc.partition_id_tensor else None
    in_names, out_names, out_avals = [], [], []
    for alloc in nc.m.functions[0].allocations:
        if not isinstance(alloc, mybir.MemoryLocationSet):
            continue
        name = alloc.memorylocations[0].name
        if alloc.kind == "ExternalInput":
            if name != partition_name:
                in_names.append(name)
        elif alloc.kind == "ExternalOutput":
            shape = tuple(alloc.tensor_shape)
            dtype = mybir.dt.np(alloc.dtype)
            out_names.append(name)
            out_avals.append(jax.core.ShapedArray(shape, dtype))
    n_params = len(in_names)
    n_outs = len(out_avals)
    all_names = in_names + out_names
    if partition_name is not None:
        all_names = all_names + [partition_name]
    donate = tuple(range(n_params, n_params + n_outs))

    def _jit_body(*args):
        operands = list(args)
        if partition_name is not None:
            operands.append(bass2jax.partition_id_tensor())
        outs = bass2jax._bass_exec_p.bind(
            *operands,
            out_avals=tuple(out_avals),
            in_names=tuple(all_names),
            out_names=tuple(out_names),
            lowering_input_output_aliases=(),
            sim_require_finite=True,
            sim_require_nnan=True,
            nc=nc,
        )
        return tuple(outs)

    devices = jax.devices()[:NCORES]
    mesh = Mesh(np.asarray(devices), ("core",))
    sharded = jax.jit(
        shard_map(
            _jit_body, mesh=mesh,
            in_specs=(PartitionSpec("core"),) * (n_params + n_outs),
            out_specs=(PartitionSpec("core"),) * n_outs,
            check_rep=False,
        ),
        donate_argnums=donate, keep_unused=True,
    )

    from jax.sharding import NamedSharding
    core_sharding = NamedSharding(mesh, PartitionSpec("core"))

    @partial(jax.jit, out_shardings=core_sharding)
    def _zeros():
        return tuple(
            jnp.zeros((NCORES * a.shape[0],) + a.shape[1:], a.dtype)
            for a in out_avals)

    def run(in_maps, device_arrays=None, timeit=False):
        import time as _time
        if device_arrays is None:
            concat_in = [
                np.concatenate([np.asarray(m[name]) for m in in_maps], axis=0)
                for name in in_names]
            device_arrays = [jax.device_put(a, core_sharding) for a in concat_in]
        zs = jax.block_until_ready(_zeros())
        t0 = _time.perf_counter()
        out_arrs = jax.block_until_ready(sharded(*device_arrays, *zs))
        dt = _time.perf_counter() - t0
        results = [
            {name: np.asarray(out_arrs[i]).reshape(NCORES, *out_avals[i].shape)[c]
             for i, name in enumerate(out_names)}
            for c in range(NCORES)]
        if timeit:
            return results, dt, device_arrays
        return results

    def bench(in_maps, iters=10, batches=3):
        """Pipelined timing: dispatch `iters` executions back-to-back and
        block once, amortizing the per-dispatch RPC latency.  Returns the
        min per-iter average across `batches` batches."""
        import time as _time
        concat_in = [
            np.concatenate([np.asarray(m[name]) for m in in_maps], axis=0)
            for name in in_names]
        device_arrays = [jax.device_put(a, core_sharding) for a in concat_in]
        best = None
        for b in range(batches):
            all_zs = [jax.block_until_ready(_zeros()) for _ in range(iters + 1)]
            jax.block_until_ready(sharded(*device_arrays, *all_zs[0]))
            t0 = _time.perf_counter()
            outs = [sharded(*device_arrays, *all_zs[1 + i]) for i in range(iters)]
            jax.block_until_ready(outs)
            dt = (_time.perf_counter() - t0) / iters
            best = dt if best is None else min(best, dt)
        return best

    _CACHE[("bench", reps)] = bench
    _CACHE[("runner", reps)] = run
    return run


def _core_inputs(x, w_qkv, b_qkv, w_out):
    """Host-side sharding: returns the 8 per-core input dicts (fp16)."""
    f16 = np.float16
    in_maps = []
    for c in range(NCORES):
        b, g = divmod(c, 4)
        e0 = g * HPC * HD  # first column of this core's head group
        q_cols = slice(e0, e0 + E_V)
        k_cols = slice(D + e0, D + e0 + E_V)
        v_cols = slice(2 * D + e0, 2 * D + e0 + E_V)
        w_qk_c = np.ascontiguousarray(
            np.concatenate([w_qkv[:, q_cols], w_qkv[:, k_cols]], axis=1)).astype(f16)
        b_qk_c = np.ascontiguousarray(
            np.concatenate([b_qkv[q_cols], b_qkv[k_cols]]).reshape(4, 128).T)
        in_maps.append({
            "xT": np.ascontiguousarray(x[b].T).astype(f16),
            "w_qk": w_qk_c,
            "w_v": np.ascontiguousarray(w_qkv[:, v_cols]).astype(f16),
            "w_o": np.ascontiguousarray(w_out[e0:e0 + E_V, :]).astype(f16),
            "b_qk": b_qk_c.astype(np.float32),
            "b_v": np.ascontiguousarray(b_qkv[v_cols]).reshape(1, E_V).astype(f16),
            "ones": np.ones((128, 128), dtype=f16),
            "onesf": np.ones((65, 512), dtype=np.float32),
        })
    return in_maps


def kernel(x, w_qkv, b_qkv, w_out, b_out):
    x = np.asarray(x, dtype=np.float32)
    w_qkv = np.asarray(w_qkv, dtype=np.float32)
    b_qkv = np.asarray(b_qkv, dtype=np.float32)
    w_out = np.asarray(w_out, dtype=np.float32)
    b_out = np.asarray(b_out, dtype=np.float32)

    run = _get_runner()
    in_maps = _core_inputs(x, w_qkv, b_qkv, w_out)
    results = run(in_maps)
    partials = np.stack([results[c]["out"].astype(np.float32) for c in range(NCORES)])
    full = partials.reshape(B, 4, S, D).sum(axis=1) + b_out
    return full.astype(np.float32)
